# revision 19
# baseline (speedup 1.0000x reference)
"""Chamfer distance kernel for 8 Trainium2 NeuronCores.

Problem: template [4, 8192, 3], source [4, 8192, 3] (fp32)
  d[b,n,m] = ||template[b,n] - source[b,m]||^2
  out[b] = mean_n min_m d + mean_m min_n d            (shape [4], fp32)

Sharding: 8 cores = 4 batches x 2 template-halves. Each core computes its
4096x8192 block of the distance matrix ONCE on the TensorEngine (augmented
K=18 matmul: d = n0 + n1 - 2<t,s>, with bf16 hi/lo coordinate splits so
every product is exact in fp32 PSUM accumulation), and reduces it in both
directions:
  - ScalarE converts each PSUM tile to a bf16 SBUF row-panel (it is the
    only engine that can read PSUM while DVE does the min work).
  - col-min partials: DVE tensor_tensor min accumulators (bf16 2x mode),
    partition-reduced at the end through PE transposes + DVE reduces.
  - row-mins: in-place log2 halving chain of TT-mins on the row panel,
    then one small reduce.
Host combines: d01 from row-min sums, d10 from elementwise min of the two
halves' col-min vectors.
"""

import numpy as np
import ml_dtypes

BF = ml_dtypes.bfloat16

B = 4
NPTS = 8192  # template points per batch
MPTS = 8192  # source points per batch
NCORES = 8
NT = NPTS // 2  # template rows per core (half batch)
K = 18  # augmented contraction slots
PTILE = 128  # row tile (PSUM partitions)
CW = 2048  # ScalarE copy width (4 PSUM banks per psum tile)
NCP = MPTS // CW  # 2 copies per row tile
GW = 2048  # colacc accumulator width
NG = MPTS // GW  # 4 column groups
NROW = NT // PTILE  # 32 row tiles
NCOLK = MPTS // PTILE  # 64 columns of colmins output
HALVE_STOP = 512  # stop the halving chain here, reduce the rest

_BIG = 3.0e38


def _bf16_parts(x64, n):
    """Split float64 array into n bf16 terms; sum of terms ~= x64."""
    parts = []
    r = np.array(x64, dtype=np.float64, copy=True)
    for _ in range(n):
        p = r.astype(BF)
        parts.append(p)
        r -= p.astype(np.float64)
    return parts


def _prep_core(templ_half, source):
    """Build the [K, NT] and [K, MPTS] bf16 slot matrices for one core.

    Slot layout (template side . source side):
      per coord c: (xh, xh, xl, xl) . (-2yh, -2yl, -2yh, -2yl)   -> 12 slots
      n0 (3-way split) . (1, 1, 1)                                -> 3 slots
      (1, 1, 1) . n1 (3-way split)                                -> 3 slots
    so sum_k ta[k,n]*sa[k,m] = ||t~_n - s~_m||^2 (t~, s~ = 16-bit-split
    coordinates; all bf16 products are exact in fp32 accumulation).
    """
    nt = templ_half.shape[0]
    ms = source.shape[0]
    t = templ_half.astype(np.float64)
    s = source.astype(np.float64)
    ta = np.zeros((K, nt), dtype=BF)
    sa = np.zeros((K, ms), dtype=BF)
    t_eff = np.zeros_like(t)
    s_eff = np.zeros_like(s)
    k = 0
    for c in range(3):
        xh, xl = _bf16_parts(t[:, c], 2)
        yh, yl = _bf16_parts(s[:, c], 2)
        t_eff[:, c] = xh.astype(np.float64) + xl.astype(np.float64)
        s_eff[:, c] = yh.astype(np.float64) + yl.astype(np.float64)
        m2yh = (-2.0 * yh.astype(np.float64)).astype(BF)  # exact (x2 = exp+1)
        m2yl = (-2.0 * yl.astype(np.float64)).astype(BF)
        ta[k + 0], sa[k + 0] = xh, m2yh
        ta[k + 1], sa[k + 1] = xh, m2yl
        ta[k + 2], sa[k + 2] = xl, m2yh
        ta[k + 3], sa[k + 3] = xl, m2yl
        k += 4
    n0 = (t_eff**2).sum(axis=1)
    n1 = (s_eff**2).sum(axis=1)
    ones_t = np.ones(nt, dtype=BF)
    ones_s = np.ones(ms, dtype=BF)
    for part in _bf16_parts(n0, 3):
        ta[k], sa[k] = part, ones_s
        k += 1
    for part in _bf16_parts(n1, 3):
        ta[k], sa[k] = ones_t, part
        k += 1
    assert k == K
    return ta, sa


def _build_bass(gpsimd_frac=0):
    """gpsimd_frac: out of 4, how many row-tiles per 4 have their colacc
    updates run on GpSimd instead of DVE (load balancing experiment)."""
    from contextlib import ExitStack

    import concourse.bacc as bacc
    import concourse.tile as tile
    from concourse import mybir

    f32 = mybir.dt.float32
    bf16 = mybir.dt.bfloat16
    MIN = mybir.AluOpType.min

    nc = bacc.Bacc("TRN2", target_bir_lowering=False)
    ta = nc.dram_tensor("ta", [K, NT], bf16, kind="ExternalInput")
    sa = nc.dram_tensor("sa", [K, MPTS], bf16, kind="ExternalInput")
    ident = nc.dram_tensor("ident", [PTILE, PTILE], bf16, kind="ExternalInput")
    rowmins = nc.dram_tensor("rowmins", [PTILE, NROW], f32, kind="ExternalOutput")
    colmins = nc.dram_tensor("colmins", [PTILE, NCOLK], f32, kind="ExternalOutput")

    with tile.TileContext(nc) as tc, ExitStack() as ctx:
        consts = ctx.enter_context(tc.tile_pool(name="consts", bufs=1))
        accs = ctx.enter_context(tc.tile_pool(name="accs", bufs=1))
        dpool = ctx.enter_context(tc.tile_pool(name="dpool", bufs=3))
        pspool = ctx.enter_context(tc.tile_pool(name="ps", bufs=2, space="PSUM"))

        # Inputs replicated at partition offsets 0 and 32 so matmuls can
        # alternate PE row-groups: LDWEIGHTS for row-group g can be pulled
        # ahead of an in-flight matmul in the other row-group, and the two
        # matmuls run concurrently in different 32-row strips of the array.
        ta_s = consts.tile([32 + K, NT], bf16, name="ta_s", tag="ta_s")
        nc.sync.dma_start(out=ta_s[0:K, :], in_=ta[:, :])
        nc.scalar.dma_start(out=ta_s[32 : 32 + K, :], in_=ta[:, :])
        sa_s = consts.tile([32 + K, MPTS], bf16, name="sa_s", tag="sa_s")
        nc.gpsimd.dma_start(out=sa_s[0:K, :], in_=sa[:, :])
        nc.gpsimd.dma_start(out=sa_s[32 : 32 + K, :], in_=sa[:, :])
        id_s = consts.tile([PTILE, PTILE], bf16, name="id_s", tag="id_s")
        nc.sync.dma_start(out=id_s, in_=ident[:, :])

        colacc = accs.tile([PTILE, MPTS], bf16, name="colacc", tag="colacc")
        rowmins_s = accs.tile([PTILE, NROW], f32, name="rowmins_s", tag="rowmins_s")
        colmins_s = accs.tile([PTILE, NCOLK], f32, name="colmins_s", tag="colmins_s")

        for i in range(NROW):
            d = dpool.tile([PTILE, MPTS], bf16, name="d", tag="d")
            eng = nc.gpsimd if (i % 4) < gpsimd_frac and i > 0 else nc.vector
            for cp in range(NCP):
                ps = pspool.tile([PTILE, CW], f32, name="ps", tag="ps")
                for q in range(CW // 512):
                    col0 = cp * CW + q * 512
                    rg = 32 * ((cp * (CW // 512) + q) % 2)
                    nc.tensor.matmul(
                        ps[:, q * 512 : (q + 1) * 512],
                        ta_s[rg : rg + K, i * PTILE : (i + 1) * PTILE],
                        sa_s[rg : rg + K, col0 : col0 + 512],
                        start=True,
                        stop=True,
                        tile_position=(rg, 0),
                    )
                dsl = slice(cp * CW, (cp + 1) * CW)
                nc.scalar.copy(d[:, dsl], ps)
                if cp % 2 == 1:
                    # Column direction: min-accumulate the completed 4096
                    # half (starts as soon as its two copies land).
                    hsl = slice((cp - 1) * CW, (cp + 1) * CW)
                    if i == 0:
                        nc.vector.tensor_copy(colacc[:, hsl], d[:, hsl])
                    else:
                        eng.tensor_tensor(
                            out=colacc[:, hsl],
                            in0=d[:, hsl],
                            in1=colacc[:, hsl],
                            op=MIN,
                        )
                    # Row direction, first halving level: fold the odd
                    # 2048-slice into the even one.
                    lo = slice((cp - 1) * CW, cp * CW)
                    nc.vector.tensor_tensor(
                        out=d[:, lo], in0=d[:, lo], in1=d[:, dsl], op=MIN
                    )
            # Row direction: remaining halving chain on d[:, :CW*2-folded]
            nc.vector.tensor_tensor(
                out=d[:, :CW], in0=d[:, :CW], in1=d[:, 2 * CW : 3 * CW], op=MIN
            )
            w = CW // 2
            while w >= HALVE_STOP:
                nc.vector.tensor_tensor(
                    out=d[:, :w], in0=d[:, :w], in1=d[:, w : 2 * w], op=MIN
                )
                w //= 2
            nc.vector.tensor_reduce(
                out=rowmins_s[:, i : i + 1],
                in_=d[:, : 2 * w],
                axis=mybir.AxisListType.X,
                op=MIN,
            )

        # Partition-reduce the column accumulators: PE transpose 128x128
        # blocks into PSUM (as bf16 slices of the fp32 pool tiles, one per
        # 2KB bank), then DVE segmented min-reduce (3D AP, axis X).
        kk = 0
        nper = CW // 512  # transposes per psum tile (one per bank)
        for t0 in range(0, NCOLK, nper):
            ps = pspool.tile([PTILE, CW], f32, name="ps", tag="ps")
            psb = ps.bitcast(bf16)  # [128, 2*CW] bf16 view
            for u in range(nper):
                t = t0 + u  # source block index: points 128*t .. 128*t+127
                nc.tensor.transpose(
                    psb[:, u * 1024 : u * 1024 + PTILE],
                    colacc[:, t * PTILE : (t + 1) * PTILE],
                    id_s,
                )
            seg = psb.rearrange("p (n x) -> p n x", x=1024)[:, :, :PTILE]
            nc.vector.tensor_reduce(
                out=colmins_s[:, kk : kk + nper],
                in_=seg,
                axis=mybir.AxisListType.X,
                op=MIN,
            )
            kk += nper
        assert kk == NCOLK

        nc.sync.dma_start(out=rowmins[:, :], in_=rowmins_s)
        nc.sync.dma_start(out=colmins[:, :], in_=colmins_s)
    nc.compile()
    return nc


_NC_CACHE = {}


import os


def _get_nc():
    if "nc" not in _NC_CACHE:
        frac = int(os.environ.get("KERNEL_GPSIMD_FRAC", "0"))
        _NC_CACHE["nc"] = _build_bass(gpsimd_frac=frac)
    return _NC_CACHE["nc"]


def kernel(template, source, _trace=False):
    from concourse.bass_utils import run_bass_kernel_spmd

    template = np.asarray(template)
    source = np.asarray(source)
    assert template.shape == (B, NPTS, 3) and source.shape == (B, MPTS, 3)

    eye = np.eye(PTILE, dtype=BF)
    in_maps = []
    for core in range(NCORES):
        b, h = core // 2, core % 2
        ta, sa = _prep_core(template[b, h * NT : (h + 1) * NT], source[b])
        in_maps.append({"ta": ta, "sa": sa, "ident": eye})

    nc = _get_nc()
    res = run_bass_kernel_spmd(
        nc, in_maps, core_ids=list(range(NCORES)), trace=_trace
    )
    results = res.results

    out = np.zeros(B, dtype=np.float64)
    for b in range(B):
        r0, r1 = results[2 * b], results[2 * b + 1]
        d01 = (
            r0["rowmins"].astype(np.float64).sum()
            + r1["rowmins"].astype(np.float64).sum()
        ) / float(NPTS)
        c0 = r0["colmins"].T.reshape(-1)  # [MPTS], source idx = 128*k + p
        c1 = r1["colmins"].T.reshape(-1)
        d10 = np.minimum(c0, c1).astype(np.float64).mean()
        out[b] = d01 + d10
    if _trace:
        kernel._last_results = res
    return out.astype(np.float32)
